# revision 13
# baseline (speedup 1.0000x reference)
"""GQA attention (B=2,S=2048,D=2048,H=16,KV=4,HD=128) + RoPE on 8 TRN2 NeuronCores.

Sharding: core c -> (batch b=c//4, kv-group g=c%4). Each core projects
Q (4 heads), K/V (1 kv head) for its batch from a replicated x^T, applies
RoPE, runs causal flash attention (scores^T layout, no-max softmax --
|scores|<9 so fp32 exp is safe), AllGathers the per-head attention outputs
across the 4-core batch group, and computes a column slice of the output
projection (column-parallel wo).

RoPE uses a de-interleaved head basis (host permutes wq/wk rows so real
parts occupy partitions 0-63 and imag parts 64-127 of each head): the
pair-swap then becomes two half-height DVE multiplies reading the PSUM
projection at a partition offset -- no PE pswap matmul, no ACT copy.

Host-side prep (inside kernel()): transpose/cast inputs to bf16, expand
RoPE tables, build identity/mask constants. Host-side post: transpose +
concatenate the 8 output column-slices.
"""
import numpy as np
import ml_dtypes

import concourse.bass as bass
import concourse.mybir as mybir
import concourse.tile as tile
from concourse import bacc
from concourse.bass import ts
from concourse.bass_utils import run_bass_kernel_spmd

BF = mybir.dt.bfloat16
F32 = mybir.dt.float32
bf16 = ml_dtypes.bfloat16

B, S, D = 2, 2048, 2048
H, KV, HD = 16, 4, 128
NT = 4          # 512-token chunks
ND = 16         # 128-wide D chunks
NH = 4          # heads per core
SCALE = 1.0 / np.sqrt(HD)
RG = [[0, 1, 2, 3], [4, 5, 6, 7]]


def build_nc():
    nc = bacc.Bacc("TRN2", target_bir_lowering=False, debug=False, num_devices=8)
    xt_d = nc.dram_tensor("xt", [D, S], BF, kind="ExternalInput").ap()
    wqkv_d = nc.dram_tensor("wqkvT", [6, 128, 2048], BF, kind="ExternalInput").ap()
    woT_d = nc.dram_tensor("woT", [D, 512], BF, kind="ExternalInput").ap()
    cos_d = nc.dram_tensor("cose", [128, S], BF, kind="ExternalInput").ap()
    sin_d = nc.dram_tensor("sins", [128, S], BF, kind="ExternalInput").ap()
    mask_d = nc.dram_tensor("mask01", [128, 896], BF, kind="ExternalInput").ap()
    ident_d = nc.dram_tensor("ident", [128, 128], BF, kind="ExternalInput").ap()
    onesc_d = nc.dram_tensor("onesc", [128, 128], BF, kind="ExternalInput").ap()
    out_d = nc.dram_tensor("out", [512, S], F32, kind="ExternalOutput").ap()

    xt_r = xt_d.rearrange("(o p) t -> p o t", p=128)      # [128, 16, 2048]
    woT_r = woT_d.rearrange("(o p) m -> p o m", p=128)    # [128, 16, 512]

    with tile.TileContext(nc) as tc:
        with (
            tc.tile_pool(name="consts", bufs=1) as consts,
            tc.tile_pool(name="io", bufs=2) as io,
            tc.tile_pool(name="work", bufs=3) as work,
            tc.tile_pool(name="psS", bufs=3, space="PSUM") as psS,
            tc.tile_pool(name="psA", bufs=3, space="PSUM") as psA,
            tc.tile_pool(name="psB", bufs=2, space="PSUM") as psB,
            tc.tile_pool(name="dram", bufs=1, space="DRAM") as dram,
        ):
            # ---- persistent SBUF; DMA emit order = availability order.
            # gpsimd queue order: ident (warmup dep) -> w_sb halves -> rope
            # tables -> attention consts. First proj MM needs only ident+
            # w_sb[0][:8]+xt q0.
            ident_sb = consts.tile([128, 128], BF, name="ident_sb")
            nc.gpsimd.dma_start(ident_sb, ident_d)
            w_sb = consts.tile([128, 6, ND, 128], BF, name="w_sb")
            for m in (4, 5, 0, 1, 2, 3):  # match proj consumption order
                for hf in range(2):
                    nc.gpsimd.dma_start(
                        w_sb[:, m, ts(hf, 8)],
                        wqkv_d[m, :, ts(hf, 1024)].rearrange(
                            "p (o c) -> p o c", c=128))
            cos_sb = consts.tile([128, S], BF, name="cos_sb")
            nc.gpsimd.dma_start(cos_sb, cos_d)
            sin_sb = consts.tile([128, S], BF, name="sin_sb")
            nc.gpsimd.dma_start(sin_sb, sin_d)
            mask_sb = consts.tile([128, 896], BF, name="mask_sb")
            nc.gpsimd.dma_start(mask_sb, mask_d)
            onesc_sb = consts.tile([128, 128], BF, name="onesc_sb")
            nc.gpsimd.dma_start(onesc_sb, onesc_d)

            # PE warmup: keep the tensor engine busy from engine-start (the
            # memset source has no DMA dependency) so the HAM clock-gate
            # opens (1.2->2.4GHz) before real matmuls arrive, and stays open
            # across the initial xt/w DMA wait.
            warm_src = consts.tile([128, 128], BF, name="warm_src")
            nc.vector.memset(warm_src, 0.0)
            warm_ps = psB.tile([128, 128], F32, tag="psB", name="warm")
            for _ in range(60):
                nc.tensor.matmul(warm_ps, lhsT=warm_src, rhs=warm_src,
                                 start=True, stop=True)

            qt_sb = consts.tile([128, NH, S], BF, name="qt_sb")   # Q^T, rope'd
            kt_sb = consts.tile([128, S], BF, name="kt_sb")       # K^T, rope'd
            v_sb = consts.tile([128, ND, HD], BF, name="v_sb")    # V [tok, hd] blocks

            ag_in = [[dram.tile([256, 512], BF, name=f"agin{i}_{p}")
                      for p in range(2)] for i in range(NT)]
            ag_out = [[dram.tile([1024, 512], BF, name=f"agout{i}_{p}")
                       for p in range(2)] for i in range(NT)]

            def proj_chunk(tc_i):
                xt_t = io.tile([128, ND, 512], BF, tag="io512", name="xt_t")
                if tc_i == 0:
                    # fine-grained d-pair DMAs alternating sync/scalar so the
                    # d-ordered proj consumption is never starved at startup
                    for q in range(8):
                        eng = nc.sync if q % 2 == 0 else nc.scalar
                        eng.dma_start(xt_t[:, 2 * q:2 * (q + 1), :],
                                      xt_r[:, 2 * q:2 * (q + 1), ts(tc_i, 512)])
                else:
                    for q in range(4):
                        nc.sync.dma_start(
                            xt_t[:, 4 * q:4 * (q + 1), :],
                            xt_r[:, 4 * q:4 * (q + 1), ts(tc_i, 512)])
                for m in (4, 5, 0, 1, 2, 3):  # K, V first: their RoPE/transpose
                    # chains overlap the Q projections, so attention never
                    # waits on kt/v_sb
                    ps = psA.tile([128, 512], F32, tag="psA", name="ps_proj")
                    for d in range(ND):
                        nc.tensor.matmul(
                            ps, lhsT=w_sb[:, m, d, :], rhs=xt_t[:, d, :],
                            start=(d == 0), stop=(d == ND - 1),
                        )
                    if m < 5:
                        # RoPE in the de-interleaved basis:
                        #   out = ps*cos + crossswap(ps)*sin_signed
                        # crossswap reads ps at partition offset +-64.
                        t1 = work.tile([128, 512], F32, tag="rope_t1", name="t1")
                        nc.vector.tensor_tensor(
                            t1, ps, cos_sb[:, ts(tc_i, 512)], mybir.AluOpType.mult)
                        t2 = work.tile([128, 512], F32, tag="rope_t2", name="t2")
                        nc.vector.tensor_tensor(
                            t2[:64], ps[64:], sin_sb[:64, ts(tc_i, 512)],
                            mybir.AluOpType.mult)
                        nc.vector.tensor_tensor(
                            t2[64:], ps[:64], sin_sb[64:, ts(tc_i, 512)],
                            mybir.AluOpType.mult)
                        dst = (qt_sb[:, m, ts(tc_i, 512)] if m < 4
                               else kt_sb[:, ts(tc_i, 512)])
                        nc.vector.tensor_tensor(dst, t1, t2, mybir.AluOpType.add)
                    else:
                        # V^T chunk -> bf16 -> transpose to [tok, hd] blocks
                        vraw = work.tile([128, 512], BF, tag="rope_raw", name="vraw")
                        nc.scalar.copy(vraw, ps)
                        for j in range(4):
                            pst = psB.tile([128, 128], BF, tag="psB", name="ps_vT")
                            nc.tensor.transpose(pst, vraw[:, ts(j, 128)], ident_sb)
                            nc.scalar.copy(v_sb[:, 4 * tc_i + j, :], pst)

            def delay(bi, off=150):
                # push off the critical path: the Tile scheduler orders by
                # bass_priority (virtual program position)
                bi.ins.bass_priority += off
                return bi

            def attn_chunk(qc):
                for h in range(NH):
                    ps_att = psB.tile([128, 512], F32, tag="psB", name="ps_att")
                    # denominator rides two bf16 elementwise accumulation
                    # chains (DVE: even kb, GpSimd: odd kb) instead of a
                    # per-block ones-matmul -- the partition reduction
                    # happens in ONE matmul per head on the merged sum.
                    pa = work.tile([128, 512], BF, tag="pa", name="pa")
                    pb = work.tile([128, 512], BF, tag="pb", name="pb")
                    nc.gpsimd.memset(pb, 0.0)
                    nkb = 4 * qc + 4
                    for kb in range(nkb):
                        r = kb - 4 * qc
                        o = max(r, 0) * 128   # first q column this kb can see
                        ps_s = psS.tile([128, 512], F32, tag="psS", name="ps_s")
                        nc.tensor.matmul(
                            ps_s[:, o:], lhsT=kt_sb[:, ts(kb, 128)],
                            rhs=qt_sb[:, h, 512 * qc + o:512 * (qc + 1)],
                            start=True, stop=True)
                        pt = work.tile([128, 512], BF, tag="pt", name="pt",
                                       bufs=8)
                        nc.scalar.activation(
                            pt[:, o:], ps_s[:, o:],
                            mybir.ActivationFunctionType.Exp, scale=SCALE)
                        if r >= 0:  # causal 0/1 mask on the hull, post-exp
                            nc.vector.tensor_tensor(
                                pt[:, o:], pt[:, o:],
                                mask_sb[:, 384:896 - o],
                                mybir.AluOpType.mult)
                        nc.tensor.matmul(
                            ps_att[:, o:], lhsT=v_sb[:, kb, :], rhs=pt[:, o:],
                            start=(kb == 0), stop=(kb == nkb - 1))
                        if kb == 0:   # kb 0 is always full-width
                            delay(nc.vector.tensor_copy(pa, pt))
                        elif kb % 2 == 0:
                            delay(nc.vector.tensor_tensor(
                                pa[:, o:], pa[:, o:], pt[:, o:],
                                mybir.AluOpType.add))
                        else:
                            nc.gpsimd.tensor_tensor(
                                pb[:, o:], pb[:, o:], pt[:, o:],
                                mybir.AluOpType.add)
                    ptb = work.tile([128, 512], BF, tag="ptb", name="ptb")
                    nc.vector.tensor_tensor(ptb, pa, pb, mybir.AluOpType.add)
                    ps_den = psA.tile([128, 512], F32, tag="psA", name="ps_den")
                    nc.tensor.matmul(ps_den, lhsT=onesc_sb, rhs=ptb,
                                     start=True, stop=True)
                    # ones[128,128] lhsT made ps_den the partition-broadcast den
                    bden = work.tile([128, 512], F32, tag="bden", name="bden")
                    nc.vector.reciprocal_approx_fast(bden, ps_den)
                    att = work.tile([128, 512], BF, tag="att", name="att")
                    nc.vector.tensor_tensor(att, ps_att, bden,
                                            mybir.AluOpType.mult)
                    nc.sync.dma_start(ag_in[qc][h // 2][ts(h % 2, 128), :], att)
                    if h % 2 == 1:
                        nc.gpsimd.collective_compute(
                            "AllGather", mybir.AluOpType.bypass,
                            replica_groups=RG,
                            ins=[ag_in[qc][h // 2][:].opt()],
                            outs=[ag_out[qc][h // 2][:].opt()])

            def oproj_chunk(tc_i):
                rhs = io.tile([128, ND, 512], BF, tag="io512", name="oproj_rhs")
                nc.sync.dma_start(
                    rhs[:, :8, :],
                    ag_out[tc_i][0].rearrange("(o p) t -> p o t", p=128))
                nc.sync.dma_start(
                    rhs[:, 8:, :],
                    ag_out[tc_i][1].rearrange("(o p) t -> p o t", p=128))
                for j in range(4):
                    if tc_i == NT - 1 and j == 3:
                        # split the very last output tile in two so the copy
                        # and store of the first half overlap the second
                        # half's matmuls (shorter kernel tail)
                        for hf in range(2):
                            ps_o = psA.tile([128, 256], F32, tag="psA",
                                            name="ps_oh")
                            for c in range(ND):
                                nc.tensor.matmul(
                                    ps_o, lhsT=woT_sb[:, c, ts(j, 128)],
                                    rhs=rhs[:, c, ts(hf, 256)],
                                    start=(c == 0), stop=(c == ND - 1))
                            o32h = work.tile([128, 256], F32, tag="o32",
                                             name="o32h")
                            nc.scalar.copy(o32h, ps_o)
                            nc.sync.dma_start(
                                out_d[ts(j, 128), 512 * tc_i + 256 * hf:
                                      512 * tc_i + 256 * (hf + 1)], o32h)
                        continue
                    ps_o = psA.tile([128, 512], F32, tag="psA", name="ps_o")
                    for c in range(ND):
                        nc.tensor.matmul(
                            ps_o, lhsT=woT_sb[:, c, ts(j, 128)], rhs=rhs[:, c, :],
                            start=(c == 0), stop=(c == ND - 1))
                    o32 = work.tile([128, 512], F32, tag="o32", name="o32")
                    nc.scalar.copy(o32, ps_o)
                    nc.sync.dma_start(out_d[ts(j, 128), ts(tc_i, 512)], o32)

            for i in range(NT):
                proj_chunk(i)
                attn_chunk(i)
            woT_sb = consts.tile([128, ND, 512], BF, name="woT_sb")
            nc.gpsimd.dma_start(woT_sb, woT_r)
            for i in range(NT):
                oproj_chunk(i)

    nc.compile()
    return nc


def make_in_maps(x, freqs_cos, freqs_sin, wq, wk, wv, wo):
    fc = np.asarray(freqs_cos, np.float32)
    fs = np.asarray(freqs_sin, np.float32)
    # De-interleaved RoPE basis: rows 0-63 real lanes, 64-127 imag lanes.
    cos_exp = np.concatenate([fc.T, fc.T], axis=0).astype(bf16)       # [128, S]
    sin_sgn = np.concatenate([-fs.T, fs.T], axis=0).astype(bf16)      # [128, S]
    mask01 = np.triu(np.ones((128, 896), np.float32), 384).astype(bf16)
    ident = np.eye(128, dtype=np.float32).astype(bf16)
    onesc = np.ones((128, 128), np.float32).astype(bf16)

    # per-head row permutation: [r0,i0,r1,i1,...] -> [r0..r63, i0..i63]
    deint = np.concatenate([np.arange(0, 128, 2), np.arange(1, 128, 2)])

    xt = [np.ascontiguousarray(np.asarray(x[b], np.float32).T).astype(bf16)
          for b in range(B)]
    wq_f = np.asarray(wq, np.float32)
    wk_f = np.asarray(wk, np.float32)
    wv_f = np.asarray(wv, np.float32)
    in_maps = []
    for core in range(8):
        b, g = divmod(core, 4)
        wq_g = wq_f[512 * g:512 * (g + 1)].reshape(4, 128, D)[:, deint, :]
        wq_g = wq_g.reshape(512, D)
        wk_g = wk_f[128 * g:128 * (g + 1)][deint, :]
        wqkvT = np.concatenate(
            [wq_g.T, wk_g.T, wv_f[128 * g:128 * (g + 1)].T], axis=1)
        # m-major SBUF-order blocks: [6][p 128][o*128+c 2048]
        wqkvT = np.ascontiguousarray(
            wqkvT.reshape(16, 128, 768).transpose(2, 1, 0)   # [768 m, 128 p, 16 o]
        )  # temp
        wqkvT = np.ascontiguousarray(np.stack(
            [wqkvT[128 * m:128 * (m + 1)].transpose(1, 2, 0).reshape(128, 2048)
             for m in range(6)]))
        order = [0, 1, 4, 5, 8, 9, 12, 13, 2, 3, 6, 7, 10, 11, 14, 15]
        woT = np.asarray(wo, np.float32)[512 * g:512 * (g + 1), :].T
        woT = woT.reshape(16, 128, 512)[order].reshape(2048, 512)
        in_maps.append({
            "xt": xt[b],
            "wqkvT": np.ascontiguousarray(wqkvT).astype(bf16),
            "woT": np.ascontiguousarray(woT).astype(bf16),
            "cose": cos_exp,
            "sins": sin_sgn,
            "mask01": mask01,
            "ident": ident,
            "onesc": onesc,
        })
    return in_maps


_NC = None


def get_nc():
    global _NC
    if _NC is None:
        _NC = build_nc()
    return _NC


def assemble_out(results):
    out = np.zeros((B, S, D), np.float32)
    for core in range(8):
        b, g = divmod(core, 4)
        out[b, :, 512 * g:512 * (g + 1)] = results[core]["out"].T
    return out


def kernel(x, freqs_cos, freqs_sin, wq, wk, wv, wo):
    import os
    os.environ.setdefault("BASS_NEVER_TRACE", "1")  # NTFF hook absent headless
    nc = get_nc()
    in_maps = make_in_maps(x, freqs_cos, freqs_sin, wq, wk, wv, wo)
    res = run_bass_kernel_spmd(nc, in_maps, core_ids=list(range(8)))
    return assemble_out(res.results)


# revision 14
# speedup vs baseline: 1.2070x; 1.2070x over previous
"""GQA attention (B=2,S=2048,D=2048,H=16,KV=4,HD=128) + RoPE on 8 TRN2 NeuronCores.

Sharding: core c -> (batch b=c//4, kv-group g=c%4). Each core projects
Q (4 heads), K/V (1 kv head) for its batch from a replicated x^T, applies
RoPE, runs causal flash attention (scores^T layout, no-max softmax --
|scores|<9 so fp32 exp is safe), AllGathers the per-head attention outputs
across the 4-core batch group, and computes a column slice of the output
projection (column-parallel wo).

RoPE uses a de-interleaved head basis (host permutes wq/wk rows so real
parts occupy partitions 0-63 and imag parts 64-127 of each head): the
pair-swap then becomes two half-height DVE multiplies reading the PSUM
projection at a partition offset -- no PE pswap matmul, no ACT copy.

Host-side prep (inside kernel()): transpose/cast inputs to bf16, expand
RoPE tables, build identity/mask constants. Host-side post: transpose +
concatenate the 8 output column-slices.
"""
import numpy as np
import ml_dtypes

import concourse.bass as bass
import concourse.mybir as mybir
import concourse.tile as tile
from concourse import bacc
from concourse.bass import ts
from concourse.bass_utils import run_bass_kernel_spmd

BF = mybir.dt.bfloat16
F32 = mybir.dt.float32
bf16 = ml_dtypes.bfloat16

B, S, D = 2, 2048, 2048
H, KV, HD = 16, 4, 128
NT = 4          # 512-token chunks
ND = 16         # 128-wide D chunks
NH = 4          # heads per core
SCALE = 1.0 / np.sqrt(HD)
RG = [[0, 1, 2, 3], [4, 5, 6, 7]]


def build_nc():
    nc = bacc.Bacc("TRN2", target_bir_lowering=False, debug=False, num_devices=8)
    xt_d = nc.dram_tensor("xt", [D, S], BF, kind="ExternalInput").ap()
    wqkv_d = nc.dram_tensor("wqkvT", [6, 128, 2048], BF, kind="ExternalInput").ap()
    woT_d = nc.dram_tensor("woT", [D, 512], BF, kind="ExternalInput").ap()
    cos_d = nc.dram_tensor("cose", [128, S], BF, kind="ExternalInput").ap()
    sin_d = nc.dram_tensor("sins", [128, S], BF, kind="ExternalInput").ap()
    mask_d = nc.dram_tensor("mask01", [128, 896], BF, kind="ExternalInput").ap()
    ident_d = nc.dram_tensor("ident", [128, 128], BF, kind="ExternalInput").ap()
    onesc_d = nc.dram_tensor("onesc", [128, 128], BF, kind="ExternalInput").ap()
    out_d = nc.dram_tensor("out", [512, S], F32, kind="ExternalOutput").ap()

    xt_r = xt_d.rearrange("(o p) t -> p o t", p=128)      # [128, 16, 2048]
    woT_r = woT_d.rearrange("(o p) m -> p o m", p=128)    # [128, 16, 512]

    with tile.TileContext(nc) as tc:
        with (
            tc.tile_pool(name="consts", bufs=1) as consts,
            tc.tile_pool(name="io", bufs=2) as io,
            tc.tile_pool(name="work", bufs=3) as work,
            tc.tile_pool(name="psS", bufs=3, space="PSUM") as psS,
            tc.tile_pool(name="psA", bufs=3, space="PSUM") as psA,
            tc.tile_pool(name="psB", bufs=2, space="PSUM") as psB,
            tc.tile_pool(name="dram", bufs=1, space="DRAM") as dram,
        ):
            # ---- persistent SBUF; DMA emit order = availability order.
            # gpsimd queue order: ident (warmup dep) -> w_sb halves -> rope
            # tables -> attention consts. First proj MM needs only ident+
            # w_sb[0][:8]+xt q0.
            ident_sb = consts.tile([128, 128], BF, name="ident_sb")
            nc.gpsimd.dma_start(ident_sb, ident_d)
            w_sb = consts.tile([128, 6, ND, 128], BF, name="w_sb")
            for m in (4, 5, 0, 1, 2, 3):  # match proj consumption order
                for hf in range(2):
                    nc.gpsimd.dma_start(
                        w_sb[:, m, ts(hf, 8)],
                        wqkv_d[m, :, ts(hf, 1024)].rearrange(
                            "p (o c) -> p o c", c=128))
            cos_sb = consts.tile([128, S], BF, name="cos_sb")
            nc.gpsimd.dma_start(cos_sb, cos_d)
            sin_sb = consts.tile([128, S], BF, name="sin_sb")
            nc.gpsimd.dma_start(sin_sb, sin_d)
            mask_sb = consts.tile([128, 896], BF, name="mask_sb")
            nc.gpsimd.dma_start(mask_sb, mask_d)
            onesc_sb = consts.tile([128, 128], BF, name="onesc_sb")
            nc.gpsimd.dma_start(onesc_sb, onesc_d)

            # PE warmup: keep the tensor engine busy from engine-start (the
            # memset source has no DMA dependency) so the HAM clock-gate
            # opens (1.2->2.4GHz) before real matmuls arrive, and stays open
            # across the initial xt/w DMA wait.
            warm_src = consts.tile([128, 128], BF, name="warm_src")
            nc.vector.memset(warm_src, 0.0)
            warm_ps = psB.tile([128, 128], F32, tag="psB", name="warm")
            for _ in range(60):
                nc.tensor.matmul(warm_ps, lhsT=warm_src, rhs=warm_src,
                                 start=True, stop=True)

            qt_sb = consts.tile([128, NH, S], BF, name="qt_sb")   # Q^T, rope'd
            kt_sb = consts.tile([128, S], BF, name="kt_sb")       # K^T, rope'd
            v_sb = consts.tile([128, ND, HD], BF, name="v_sb")    # V [tok, hd] blocks

            ag_in = [[dram.tile([256, 512], BF, name=f"agin{i}_{p}")
                      for p in range(2)] for i in range(NT)]
            ag_out = [[dram.tile([1024, 512], BF, name=f"agout{i}_{p}")
                       for p in range(2)] for i in range(NT)]

            def proj_chunk(tc_i):
                xt_t = io.tile([128, ND, 512], BF, tag="io512", name="xt_t")
                if tc_i == 0:
                    # fine-grained d-pair DMAs alternating sync/scalar so the
                    # d-ordered proj consumption is never starved at startup
                    for q in range(8):
                        eng = nc.sync if q % 2 == 0 else nc.scalar
                        eng.dma_start(xt_t[:, 2 * q:2 * (q + 1), :],
                                      xt_r[:, 2 * q:2 * (q + 1), ts(tc_i, 512)])
                else:
                    for q in range(4):
                        nc.sync.dma_start(
                            xt_t[:, 4 * q:4 * (q + 1), :],
                            xt_r[:, 4 * q:4 * (q + 1), ts(tc_i, 512)])
                for m in (4, 5, 0, 1, 2, 3):  # K, V first: their RoPE/transpose
                    # chains overlap the Q projections, so attention never
                    # waits on kt/v_sb
                    ps = psA.tile([128, 512], F32, tag="psA", name="ps_proj")
                    for d in range(ND):
                        nc.tensor.matmul(
                            ps, lhsT=w_sb[:, m, d, :], rhs=xt_t[:, d, :],
                            start=(d == 0), stop=(d == ND - 1),
                        )
                    if m < 5:
                        # RoPE in the de-interleaved basis:
                        #   out = ps*cos + crossswap(ps)*sin_signed
                        # crossswap reads ps at partition offset +-64.
                        t1 = work.tile([128, 512], F32, tag="rope_t1", name="t1")
                        nc.vector.tensor_tensor(
                            t1, ps, cos_sb[:, ts(tc_i, 512)], mybir.AluOpType.mult)
                        t2 = work.tile([128, 512], F32, tag="rope_t2", name="t2")
                        nc.vector.tensor_tensor(
                            t2[:64], ps[64:], sin_sb[:64, ts(tc_i, 512)],
                            mybir.AluOpType.mult)
                        nc.vector.tensor_tensor(
                            t2[64:], ps[:64], sin_sb[64:, ts(tc_i, 512)],
                            mybir.AluOpType.mult)
                        dst = (qt_sb[:, m, ts(tc_i, 512)] if m < 4
                               else kt_sb[:, ts(tc_i, 512)])
                        nc.vector.tensor_tensor(dst, t1, t2, mybir.AluOpType.add)
                    else:
                        # V^T chunk -> bf16 -> transpose to [tok, hd] blocks
                        vraw = work.tile([128, 512], BF, tag="rope_raw", name="vraw")
                        nc.scalar.copy(vraw, ps)
                        for j in range(4):
                            pst = psB.tile([128, 128], BF, tag="psB", name="ps_vT")
                            nc.tensor.transpose(pst, vraw[:, ts(j, 128)], ident_sb)
                            nc.vector.tensor_copy(v_sb[:, 4 * tc_i + j, :], pst)

            def attn_chunk(qc):
                for h in range(NH):
                    ps_att = psB.tile([128, 512], F32, tag="psB", name="ps_att")
                    # denominator rides two bf16 elementwise accumulation
                    # chains (DVE: even kb, GpSimd: odd kb) instead of a
                    # per-block ones-matmul -- the partition reduction
                    # happens in ONE matmul per head on the merged sum.
                    pa = work.tile([128, 512], BF, tag="pa", name="pa")
                    pb = work.tile([128, 512], BF, tag="pb", name="pb")
                    nc.gpsimd.memset(pb, 0.0)
                    nkb = 4 * qc + 4
                    for kb in range(nkb):
                        r = kb - 4 * qc
                        o = max(r, 0) * 128   # first q column this kb can see
                        ps_s = psS.tile([128, 512], F32, tag="psS", name="ps_s")
                        nc.tensor.matmul(
                            ps_s[:, o:], lhsT=kt_sb[:, ts(kb, 128)],
                            rhs=qt_sb[:, h, 512 * qc + o:512 * (qc + 1)],
                            start=True, stop=True)
                        pt = work.tile([128, 512], BF, tag="pt", name="pt",
                                       bufs=8)
                        nc.scalar.activation(
                            pt[:, o:], ps_s[:, o:],
                            mybir.ActivationFunctionType.Exp, scale=SCALE)
                        if r >= 0:  # causal 0/1 mask on the hull, post-exp
                            nc.vector.tensor_tensor(
                                pt[:, o:], pt[:, o:],
                                mask_sb[:, 384:896 - o],
                                mybir.AluOpType.mult)
                        nc.tensor.matmul(
                            ps_att[:, o:], lhsT=v_sb[:, kb, :], rhs=pt[:, o:],
                            start=(kb == 0), stop=(kb == nkb - 1))
                        if kb == 0:   # kb 0 is always full-width
                            nc.vector.tensor_copy(pa, pt)
                        elif kb % 2 == 0:
                            nc.vector.tensor_tensor(
                                pa[:, o:], pa[:, o:], pt[:, o:],
                                mybir.AluOpType.add)
                        else:
                            nc.gpsimd.tensor_tensor(
                                pb[:, o:], pb[:, o:], pt[:, o:],
                                mybir.AluOpType.add)
                    ptb = work.tile([128, 512], BF, tag="ptb", name="ptb")
                    nc.vector.tensor_tensor(ptb, pa, pb, mybir.AluOpType.add)
                    ps_den = psA.tile([128, 512], F32, tag="psA", name="ps_den")
                    nc.tensor.matmul(ps_den, lhsT=onesc_sb, rhs=ptb,
                                     start=True, stop=True)
                    # ones[128,128] lhsT made ps_den the partition-broadcast den
                    bden = work.tile([128, 512], F32, tag="bden", name="bden")
                    nc.vector.reciprocal_approx_fast(bden, ps_den)
                    att = work.tile([128, 512], BF, tag="att", name="att")
                    nc.vector.tensor_tensor(att, ps_att, bden,
                                            mybir.AluOpType.mult)
                    nc.scalar.dma_start(ag_in[qc][h // 2][ts(h % 2, 128), :], att)
                    if h % 2 == 1:
                        nc.gpsimd.collective_compute(
                            "AllGather", mybir.AluOpType.bypass,
                            replica_groups=RG,
                            ins=[ag_in[qc][h // 2][:].opt()],
                            outs=[ag_out[qc][h // 2][:].opt()])

            def oproj_chunk(tc_i):
                rhs = io.tile([128, ND, 512], BF, tag="io512", name="oproj_rhs")
                nc.sync.dma_start(
                    rhs[:, :8, :],
                    ag_out[tc_i][0].rearrange("(o p) t -> p o t", p=128))
                nc.sync.dma_start(
                    rhs[:, 8:, :],
                    ag_out[tc_i][1].rearrange("(o p) t -> p o t", p=128))
                for j in range(4):
                    if tc_i == NT - 1 and j == 3:
                        # split the very last output tile in two so the copy
                        # and store of the first half overlap the second
                        # half's matmuls (shorter kernel tail)
                        for hf in range(2):
                            ps_o = psA.tile([128, 256], F32, tag="psA",
                                            name="ps_oh")
                            for c in range(ND):
                                nc.tensor.matmul(
                                    ps_o, lhsT=woT_sb[:, c, ts(j, 128)],
                                    rhs=rhs[:, c, ts(hf, 256)],
                                    start=(c == 0), stop=(c == ND - 1))
                            o32h = work.tile([128, 256], F32, tag="o32",
                                             name="o32h")
                            nc.vector.tensor_copy(o32h, ps_o)
                            nc.sync.dma_start(
                                out_d[ts(j, 128), 512 * tc_i + 256 * hf:
                                      512 * tc_i + 256 * (hf + 1)], o32h)
                        continue
                    ps_o = psA.tile([128, 512], F32, tag="psA", name="ps_o")
                    for c in range(ND):
                        nc.tensor.matmul(
                            ps_o, lhsT=woT_sb[:, c, ts(j, 128)], rhs=rhs[:, c, :],
                            start=(c == 0), stop=(c == ND - 1))
                    o32 = work.tile([128, 512], F32, tag="o32", name="o32")
                    nc.vector.tensor_copy(o32, ps_o)
                    nc.sync.dma_start(out_d[ts(j, 128), ts(tc_i, 512)], o32)

            for i in range(NT):
                proj_chunk(i)
                attn_chunk(i)
            woT_sb = consts.tile([128, ND, 512], BF, name="woT_sb")
            nc.gpsimd.dma_start(woT_sb, woT_r)
            for i in range(NT):
                oproj_chunk(i)

    nc.compile()
    return nc


def make_in_maps(x, freqs_cos, freqs_sin, wq, wk, wv, wo):
    fc = np.asarray(freqs_cos, np.float32)
    fs = np.asarray(freqs_sin, np.float32)
    # De-interleaved RoPE basis: rows 0-63 real lanes, 64-127 imag lanes.
    cos_exp = np.concatenate([fc.T, fc.T], axis=0).astype(bf16)       # [128, S]
    sin_sgn = np.concatenate([-fs.T, fs.T], axis=0).astype(bf16)      # [128, S]
    mask01 = np.triu(np.ones((128, 896), np.float32), 384).astype(bf16)
    ident = np.eye(128, dtype=np.float32).astype(bf16)
    onesc = np.ones((128, 128), np.float32).astype(bf16)

    # per-head row permutation: [r0,i0,r1,i1,...] -> [r0..r63, i0..i63]
    deint = np.concatenate([np.arange(0, 128, 2), np.arange(1, 128, 2)])

    xt = [np.ascontiguousarray(np.asarray(x[b], np.float32).T).astype(bf16)
          for b in range(B)]
    wq_f = np.asarray(wq, np.float32)
    wk_f = np.asarray(wk, np.float32)
    wv_f = np.asarray(wv, np.float32)
    in_maps = []
    for core in range(8):
        b, g = divmod(core, 4)
        wq_g = wq_f[512 * g:512 * (g + 1)].reshape(4, 128, D)[:, deint, :]
        wq_g = wq_g.reshape(512, D)
        wk_g = wk_f[128 * g:128 * (g + 1)][deint, :]
        wqkvT = np.concatenate(
            [wq_g.T, wk_g.T, wv_f[128 * g:128 * (g + 1)].T], axis=1)
        # m-major SBUF-order blocks: [6][p 128][o*128+c 2048]
        wqkvT = np.ascontiguousarray(
            wqkvT.reshape(16, 128, 768).transpose(2, 1, 0)   # [768 m, 128 p, 16 o]
        )  # temp
        wqkvT = np.ascontiguousarray(np.stack(
            [wqkvT[128 * m:128 * (m + 1)].transpose(1, 2, 0).reshape(128, 2048)
             for m in range(6)]))
        order = [0, 1, 4, 5, 8, 9, 12, 13, 2, 3, 6, 7, 10, 11, 14, 15]
        woT = np.asarray(wo, np.float32)[512 * g:512 * (g + 1), :].T
        woT = woT.reshape(16, 128, 512)[order].reshape(2048, 512)
        in_maps.append({
            "xt": xt[b],
            "wqkvT": np.ascontiguousarray(wqkvT).astype(bf16),
            "woT": np.ascontiguousarray(woT).astype(bf16),
            "cose": cos_exp,
            "sins": sin_sgn,
            "mask01": mask01,
            "ident": ident,
            "onesc": onesc,
        })
    return in_maps


_NC = None


def get_nc():
    global _NC
    if _NC is None:
        _NC = build_nc()
    return _NC


def assemble_out(results):
    out = np.zeros((B, S, D), np.float32)
    for core in range(8):
        b, g = divmod(core, 4)
        out[b, :, 512 * g:512 * (g + 1)] = results[core]["out"].T
    return out


def kernel(x, freqs_cos, freqs_sin, wq, wk, wv, wo):
    import os
    os.environ.setdefault("BASS_NEVER_TRACE", "1")  # NTFF hook absent headless
    nc = get_nc()
    in_maps = make_in_maps(x, freqs_cos, freqs_sin, wq, wk, wv, wo)
    res = run_bass_kernel_spmd(nc, in_maps, core_ids=list(range(8)))
    return assemble_out(res.results)


# revision 15
# speedup vs baseline: 1.3906x; 1.1521x over previous
"""GQA attention (B=2,S=2048,D=2048,H=16,KV=4,HD=128) + RoPE on 8 TRN2 NeuronCores.

Sharding: core c -> (batch b=c//4, kv-group g=c%4). Each core projects
Q (4 heads), K/V (1 kv head) for its batch from a replicated x^T, applies
RoPE, runs causal flash attention (scores^T layout, no-max softmax --
|scores|<9 so fp32 exp is safe), AllGathers the per-head attention outputs
across the 4-core batch group, and computes a column slice of the output
projection (column-parallel wo).

RoPE uses a de-interleaved head basis (host permutes wq/wk rows so real
parts occupy partitions 0-63 and imag parts 64-127 of each head): the
pair-swap then becomes two half-height DVE multiplies reading the PSUM
projection at a partition offset -- no PE pswap matmul, no ACT copy.

Host-side prep (inside kernel()): transpose/cast inputs to bf16, expand
RoPE tables, build identity/mask constants. Host-side post: transpose +
concatenate the 8 output column-slices.
"""
import numpy as np
import ml_dtypes

import concourse.bass as bass
import concourse.mybir as mybir
import concourse.tile as tile
from concourse import bacc
from concourse.bass import ts
from concourse.bass_utils import run_bass_kernel_spmd

BF = mybir.dt.bfloat16
F32 = mybir.dt.float32
bf16 = ml_dtypes.bfloat16

B, S, D = 2, 2048, 2048
H, KV, HD = 16, 4, 128
NT = 4          # 512-token chunks
ND = 16         # 128-wide D chunks
NH = 4          # heads per core
SCALE = 1.0 / np.sqrt(HD)
RG = [[0, 1, 2, 3], [4, 5, 6, 7]]


def build_nc():
    nc = bacc.Bacc("TRN2", target_bir_lowering=False, debug=False, num_devices=8)
    xt_d = nc.dram_tensor("xt", [D, S], BF, kind="ExternalInput").ap()
    wqkv_d = nc.dram_tensor("wqkvT", [6, 128, 2048], BF, kind="ExternalInput").ap()
    woT_d = nc.dram_tensor("woT", [D, 512], BF, kind="ExternalInput").ap()
    cos_d = nc.dram_tensor("cose", [128, S], BF, kind="ExternalInput").ap()
    sin_d = nc.dram_tensor("sins", [128, S], BF, kind="ExternalInput").ap()
    trim_d = nc.dram_tensor("trim", [128, 128], BF, kind="ExternalInput").ap()
    ident_d = nc.dram_tensor("ident", [128, 128], BF, kind="ExternalInput").ap()
    onesc_d = nc.dram_tensor("onesc", [128, 128], BF, kind="ExternalInput").ap()
    out_d = nc.dram_tensor("out", [512, S], F32, kind="ExternalOutput").ap()

    xt_r = xt_d.rearrange("(o p) t -> p o t", p=128)      # [128, 16, 2048]
    woT_r = woT_d.rearrange("(o p) m -> p o m", p=128)    # [128, 16, 512]

    with tile.TileContext(nc) as tc:
        with (
            tc.tile_pool(name="consts", bufs=1) as consts,
            tc.tile_pool(name="io", bufs=2) as io,
            tc.tile_pool(name="work", bufs=3) as work,
            tc.tile_pool(name="psS", bufs=3, space="PSUM") as psS,
            tc.tile_pool(name="psA", bufs=3, space="PSUM") as psA,
            tc.tile_pool(name="psB", bufs=2, space="PSUM") as psB,
            tc.tile_pool(name="dram", bufs=1, space="DRAM") as dram,
        ):
            # ---- persistent SBUF; DMA emit order = availability order.
            # gpsimd queue order: ident (warmup dep) -> w_sb halves -> rope
            # tables -> attention consts. First proj MM needs only ident+
            # w_sb[0][:8]+xt q0.
            ident_sb = consts.tile([128, 128], BF, name="ident_sb")
            nc.gpsimd.dma_start(ident_sb, ident_d)
            w_sb = consts.tile([128, 6, ND, 128], BF, name="w_sb")
            for m in (4, 5, 0, 1, 2, 3):  # match proj consumption order
                for hf in range(2):
                    nc.gpsimd.dma_start(
                        w_sb[:, m, ts(hf, 8)],
                        wqkv_d[m, :, ts(hf, 1024)].rearrange(
                            "p (o c) -> p o c", c=128))
            cos_sb = consts.tile([128, S], BF, name="cos_sb")
            nc.gpsimd.dma_start(cos_sb, cos_d)
            sin_sb = consts.tile([128, S], BF, name="sin_sb")
            nc.gpsimd.dma_start(sin_sb, sin_d)
            trim_sb = consts.tile([128, 128], BF, name="trim_sb")
            nc.gpsimd.dma_start(trim_sb, trim_d)
            onesc_sb = consts.tile([128, 128], BF, name="onesc_sb")
            nc.gpsimd.dma_start(onesc_sb, onesc_d)

            # PE warmup: keep the tensor engine busy from engine-start (the
            # memset source has no DMA dependency) so the HAM clock-gate
            # opens (1.2->2.4GHz) before real matmuls arrive, and stays open
            # across the initial xt/w DMA wait.
            warm_src = consts.tile([128, 128], BF, name="warm_src")
            nc.vector.memset(warm_src, 0.0)
            warm_ps = psB.tile([128, 128], F32, tag="psB", name="warm")
            for _ in range(60):
                nc.tensor.matmul(warm_ps, lhsT=warm_src, rhs=warm_src,
                                 start=True, stop=True)

            qt_sb = consts.tile([128, NH, S], BF, name="qt_sb")   # Q^T, rope'd
            kt_sb = consts.tile([128, S], BF, name="kt_sb")       # K^T, rope'd
            v_sb = consts.tile([128, ND, HD], BF, name="v_sb")    # V [tok, hd] blocks

            ag_in = [[dram.tile([256, 512], BF, name=f"agin{i}_{p}")
                      for p in range(2)] for i in range(NT)]
            ag_out = [[dram.tile([1024, 512], BF, name=f"agout{i}_{p}")
                       for p in range(2)] for i in range(NT)]

            def proj_chunk(tc_i):
                xt_t = io.tile([128, ND, 512], BF, tag="io512", name="xt_t")
                if tc_i == 0:
                    # fine-grained d-pair DMAs alternating sync/scalar so the
                    # d-ordered proj consumption is never starved at startup
                    for q in range(8):
                        eng = nc.sync if q % 2 == 0 else nc.scalar
                        eng.dma_start(xt_t[:, 2 * q:2 * (q + 1), :],
                                      xt_r[:, 2 * q:2 * (q + 1), ts(tc_i, 512)])
                else:
                    for q in range(4):
                        nc.sync.dma_start(
                            xt_t[:, 4 * q:4 * (q + 1), :],
                            xt_r[:, 4 * q:4 * (q + 1), ts(tc_i, 512)])
                for m in (4, 5, 0, 1, 2, 3):  # K, V first: their RoPE/transpose
                    # chains overlap the Q projections, so attention never
                    # waits on kt/v_sb
                    ps = psA.tile([128, 512], F32, tag="psA", name="ps_proj")
                    for d in range(ND):
                        nc.tensor.matmul(
                            ps, lhsT=w_sb[:, m, d, :], rhs=xt_t[:, d, :],
                            start=(d == 0), stop=(d == ND - 1),
                        )
                    if m < 5:
                        # RoPE in the de-interleaved basis:
                        #   out = ps*cos + crossswap(ps)*sin_signed
                        # crossswap reads ps at partition offset +-64.
                        t1 = work.tile([128, 512], F32, tag="rope_t1", name="t1")
                        nc.vector.tensor_tensor(
                            t1, ps, cos_sb[:, ts(tc_i, 512)], mybir.AluOpType.mult)
                        t2 = work.tile([128, 512], F32, tag="rope_t2", name="t2")
                        nc.vector.tensor_tensor(
                            t2[:64], ps[64:], sin_sb[:64, ts(tc_i, 512)],
                            mybir.AluOpType.mult)
                        nc.vector.tensor_tensor(
                            t2[64:], ps[:64], sin_sb[64:, ts(tc_i, 512)],
                            mybir.AluOpType.mult)
                        dst = (qt_sb[:, m, ts(tc_i, 512)] if m < 4
                               else kt_sb[:, ts(tc_i, 512)])
                        nc.vector.tensor_tensor(dst, t1, t2, mybir.AluOpType.add)
                    else:
                        # V^T chunk -> bf16 -> transpose to [tok, hd] blocks
                        vraw = work.tile([128, 512], BF, tag="rope_raw", name="vraw")
                        nc.scalar.copy(vraw, ps)
                        for j in range(4):
                            pst = psB.tile([128, 128], BF, tag="psB", name="ps_vT")
                            nc.tensor.transpose(pst, vraw[:, ts(j, 128)], ident_sb)
                            nc.vector.tensor_copy(v_sb[:, 4 * tc_i + j, :], pst)

            def attn_chunk(qc):
                for h in range(NH):
                    ps_att = psB.tile([128, 512], F32, tag="psB", name="ps_att")
                    # denominator rides two bf16 elementwise accumulation
                    # chains (DVE: even kb, GpSimd: odd kb) instead of a
                    # per-block ones-matmul -- the partition reduction
                    # happens in ONE matmul per head on the merged sum.
                    pa = work.tile([128, 512], BF, tag="pa", name="pa")
                    pb = work.tile([128, 512], BF, tag="pb", name="pb")
                    nc.gpsimd.memset(pb, 0.0)
                    nkb = 4 * qc + 4
                    for kb in range(nkb):
                        r = kb - 4 * qc
                        o = max(r, 0) * 128   # first q column this kb can see
                        ps_s = psS.tile([128, 512], F32, tag="psS", name="ps_s")
                        nc.tensor.matmul(
                            ps_s[:, o:], lhsT=kt_sb[:, ts(kb, 128)],
                            rhs=qt_sb[:, h, 512 * qc + o:512 * (qc + 1)],
                            start=True, stop=(r < 0))
                        if r >= 0:
                            # causal mask folded into the scores: -30/SCALE
                            # bias on the strictly-masked diagonal square;
                            # exp(s-30) ~ 0. Keeps DVE off the exp->pV path.
                            nc.tensor.matmul(
                                ps_s[:, o:o + 128], lhsT=trim_sb,
                                rhs=ident_sb, start=False, stop=True)
                        pt = work.tile([128, 512], BF, tag="pt", name="pt",
                                       bufs=8)
                        nc.scalar.activation(
                            pt[:, o:], ps_s[:, o:],
                            mybir.ActivationFunctionType.Exp, scale=SCALE)
                        nc.tensor.matmul(
                            ps_att[:, o:], lhsT=v_sb[:, kb, :], rhs=pt[:, o:],
                            start=(kb == 0), stop=(kb == nkb - 1))
                        if kb == 0:   # kb 0 is always full-width
                            nc.vector.tensor_copy(pa, pt)
                        elif kb % 2 == 0:
                            nc.vector.tensor_tensor(
                                pa[:, o:], pa[:, o:], pt[:, o:],
                                mybir.AluOpType.add)
                        else:
                            nc.gpsimd.tensor_tensor(
                                pb[:, o:], pb[:, o:], pt[:, o:],
                                mybir.AluOpType.add)
                    ptb = work.tile([128, 512], BF, tag="ptb", name="ptb")
                    nc.vector.tensor_tensor(ptb, pa, pb, mybir.AluOpType.add)
                    ps_den = psA.tile([128, 512], F32, tag="psA", name="ps_den")
                    nc.tensor.matmul(ps_den, lhsT=onesc_sb, rhs=ptb,
                                     start=True, stop=True)
                    # ones[128,128] lhsT made ps_den the partition-broadcast den
                    bden = work.tile([128, 512], F32, tag="bden", name="bden")
                    nc.vector.reciprocal_approx_fast(bden, ps_den)
                    att = work.tile([128, 512], BF, tag="att", name="att")
                    nc.vector.tensor_tensor(att, ps_att, bden,
                                            mybir.AluOpType.mult)
                    nc.scalar.dma_start(ag_in[qc][h // 2][ts(h % 2, 128), :], att)
                    if h % 2 == 1:
                        nc.gpsimd.collective_compute(
                            "AllGather", mybir.AluOpType.bypass,
                            replica_groups=RG,
                            ins=[ag_in[qc][h // 2][:].opt()],
                            outs=[ag_out[qc][h // 2][:].opt()])

            def oproj_chunk(tc_i):
                rhs = io.tile([128, ND, 512], BF, tag="io512", name="oproj_rhs")
                nc.sync.dma_start(
                    rhs[:, :8, :],
                    ag_out[tc_i][0].rearrange("(o p) t -> p o t", p=128))
                nc.sync.dma_start(
                    rhs[:, 8:, :],
                    ag_out[tc_i][1].rearrange("(o p) t -> p o t", p=128))
                for j in range(4):
                    if tc_i == NT - 1 and j == 3:
                        # split the very last output tile in two so the copy
                        # and store of the first half overlap the second
                        # half's matmuls (shorter kernel tail)
                        for hf in range(2):
                            ps_o = psA.tile([128, 256], F32, tag="psA",
                                            name="ps_oh")
                            for c in range(ND):
                                nc.tensor.matmul(
                                    ps_o, lhsT=woT_sb[:, c, ts(j, 128)],
                                    rhs=rhs[:, c, ts(hf, 256)],
                                    start=(c == 0), stop=(c == ND - 1))
                            o32h = work.tile([128, 256], F32, tag="o32",
                                             name="o32h")
                            nc.vector.tensor_copy(o32h, ps_o)
                            nc.sync.dma_start(
                                out_d[ts(j, 128), 512 * tc_i + 256 * hf:
                                      512 * tc_i + 256 * (hf + 1)], o32h)
                        continue
                    ps_o = psA.tile([128, 512], F32, tag="psA", name="ps_o")
                    for c in range(ND):
                        nc.tensor.matmul(
                            ps_o, lhsT=woT_sb[:, c, ts(j, 128)], rhs=rhs[:, c, :],
                            start=(c == 0), stop=(c == ND - 1))
                    o32 = work.tile([128, 512], F32, tag="o32", name="o32")
                    nc.vector.tensor_copy(o32, ps_o)
                    nc.sync.dma_start(out_d[ts(j, 128), ts(tc_i, 512)], o32)

            for i in range(NT):
                proj_chunk(i)
                attn_chunk(i)
            woT_sb = consts.tile([128, ND, 512], BF, name="woT_sb")
            nc.gpsimd.dma_start(woT_sb, woT_r)
            for i in range(NT):
                oproj_chunk(i)

    nc.compile()
    return nc


def make_in_maps(x, freqs_cos, freqs_sin, wq, wk, wv, wo):
    fc = np.asarray(freqs_cos, np.float32)
    fs = np.asarray(freqs_sin, np.float32)
    # De-interleaved RoPE basis: rows 0-63 real lanes, 64-127 imag lanes.
    cos_exp = np.concatenate([fc.T, fc.T], axis=0).astype(bf16)       # [128, S]
    sin_sgn = np.concatenate([-fs.T, fs.T], axis=0).astype(bf16)      # [128, S]
    trim = (np.triu(np.ones((128, 128), np.float32), 1) * (-30.0 / SCALE)).astype(bf16)
    ident = np.eye(128, dtype=np.float32).astype(bf16)
    onesc = np.ones((128, 128), np.float32).astype(bf16)

    # per-head row permutation: [r0,i0,r1,i1,...] -> [r0..r63, i0..i63]
    deint = np.concatenate([np.arange(0, 128, 2), np.arange(1, 128, 2)])

    xt = [np.ascontiguousarray(np.asarray(x[b], np.float32).T).astype(bf16)
          for b in range(B)]
    wq_f = np.asarray(wq, np.float32)
    wk_f = np.asarray(wk, np.float32)
    wv_f = np.asarray(wv, np.float32)
    in_maps = []
    for core in range(8):
        b, g = divmod(core, 4)
        wq_g = wq_f[512 * g:512 * (g + 1)].reshape(4, 128, D)[:, deint, :]
        wq_g = wq_g.reshape(512, D)
        wk_g = wk_f[128 * g:128 * (g + 1)][deint, :]
        wqkvT = np.concatenate(
            [wq_g.T, wk_g.T, wv_f[128 * g:128 * (g + 1)].T], axis=1)
        # m-major SBUF-order blocks: [6][p 128][o*128+c 2048]
        wqkvT = np.ascontiguousarray(
            wqkvT.reshape(16, 128, 768).transpose(2, 1, 0)   # [768 m, 128 p, 16 o]
        )  # temp
        wqkvT = np.ascontiguousarray(np.stack(
            [wqkvT[128 * m:128 * (m + 1)].transpose(1, 2, 0).reshape(128, 2048)
             for m in range(6)]))
        order = [0, 1, 4, 5, 8, 9, 12, 13, 2, 3, 6, 7, 10, 11, 14, 15]
        woT = np.asarray(wo, np.float32)[512 * g:512 * (g + 1), :].T
        woT = woT.reshape(16, 128, 512)[order].reshape(2048, 512)
        in_maps.append({
            "xt": xt[b],
            "wqkvT": np.ascontiguousarray(wqkvT).astype(bf16),
            "woT": np.ascontiguousarray(woT).astype(bf16),
            "cose": cos_exp,
            "sins": sin_sgn,
            "trim": trim,
            "ident": ident,
            "onesc": onesc,
        })
    return in_maps


_NC = None


def get_nc():
    global _NC
    if _NC is None:
        _NC = build_nc()
    return _NC


def assemble_out(results):
    out = np.zeros((B, S, D), np.float32)
    for core in range(8):
        b, g = divmod(core, 4)
        out[b, :, 512 * g:512 * (g + 1)] = results[core]["out"].T
    return out


def kernel(x, freqs_cos, freqs_sin, wq, wk, wv, wo):
    import os
    os.environ.setdefault("BASS_NEVER_TRACE", "1")  # NTFF hook absent headless
    nc = get_nc()
    in_maps = make_in_maps(x, freqs_cos, freqs_sin, wq, wk, wv, wo)
    res = run_bass_kernel_spmd(nc, in_maps, core_ids=list(range(8)))
    return assemble_out(res.results)


# revision 16
# speedup vs baseline: 1.4503x; 1.0429x over previous
"""GQA attention (B=2,S=2048,D=2048,H=16,KV=4,HD=128) + RoPE on 8 TRN2 NeuronCores.

Sharding: core c -> (batch b=c//4, kv-group g=c%4). Each core projects
Q (4 heads), K/V (1 kv head) for its batch from a replicated x^T, applies
RoPE, runs causal flash attention (scores^T layout, no-max softmax --
|scores|<9 so fp32 exp is safe), AllGathers the per-head attention outputs
across the 4-core batch group, and computes a column slice of the output
projection (column-parallel wo).

RoPE uses a de-interleaved head basis (host permutes wq/wk rows so real
parts occupy partitions 0-63 and imag parts 64-127 of each head): the
pair-swap then becomes two half-height DVE multiplies reading the PSUM
projection at a partition offset -- no PE pswap matmul, no ACT copy.

Host-side prep (inside kernel()): transpose/cast inputs to bf16, expand
RoPE tables, build identity/mask constants. Host-side post: transpose +
concatenate the 8 output column-slices.
"""
import numpy as np
import ml_dtypes

import concourse.bass as bass
import concourse.mybir as mybir
import concourse.tile as tile
from concourse import bacc
from concourse.bass import ts
from concourse.bass_utils import run_bass_kernel_spmd

BF = mybir.dt.bfloat16
F32 = mybir.dt.float32
bf16 = ml_dtypes.bfloat16

B, S, D = 2, 2048, 2048
H, KV, HD = 16, 4, 128
NT = 4          # 512-token chunks
ND = 16         # 128-wide D chunks
NH = 4          # heads per core
SCALE = 1.0 / np.sqrt(HD)
RG = [[0, 1, 2, 3], [4, 5, 6, 7]]


def build_nc():
    nc = bacc.Bacc("TRN2", target_bir_lowering=False, debug=False, num_devices=8)
    xt_d = nc.dram_tensor("xt", [D, S], BF, kind="ExternalInput").ap()
    wqkv_d = nc.dram_tensor("wqkvT", [6, 128, 2048], BF, kind="ExternalInput").ap()
    woT_d = nc.dram_tensor("woT", [D, 512], BF, kind="ExternalInput").ap()
    cos_d = nc.dram_tensor("cose", [128, S], BF, kind="ExternalInput").ap()
    sin_d = nc.dram_tensor("sins", [128, S], BF, kind="ExternalInput").ap()
    trim_d = nc.dram_tensor("trim", [128, 128], BF, kind="ExternalInput").ap()
    ident_d = nc.dram_tensor("ident", [128, 128], BF, kind="ExternalInput").ap()
    onesc_d = nc.dram_tensor("onesc", [128, 128], BF, kind="ExternalInput").ap()
    out_d = nc.dram_tensor("out", [512, S], F32, kind="ExternalOutput").ap()

    xt_r = xt_d.rearrange("(o p) t -> p o t", p=128)      # [128, 16, 2048]
    woT_r = woT_d.rearrange("(o p) m -> p o m", p=128)    # [128, 16, 512]

    with tile.TileContext(nc) as tc:
        with (
            tc.tile_pool(name="consts", bufs=1) as consts,
            tc.tile_pool(name="io", bufs=2) as io,
            tc.tile_pool(name="work", bufs=3) as work,
            tc.tile_pool(name="psS", bufs=3, space="PSUM") as psS,
            tc.tile_pool(name="psA", bufs=3, space="PSUM") as psA,
            tc.tile_pool(name="psB", bufs=2, space="PSUM") as psB,
            tc.tile_pool(name="dram", bufs=1, space="DRAM") as dram,
        ):
            # ---- persistent SBUF; DMA emit order = availability order.
            # gpsimd queue order: ident (warmup dep) -> w_sb halves -> rope
            # tables -> attention consts. First proj MM needs only ident+
            # w_sb[0][:8]+xt q0.
            ident_sb = consts.tile([128, 128], BF, name="ident_sb")
            nc.gpsimd.dma_start(ident_sb, ident_d)
            w_sb = consts.tile([128, 6, ND, 128], BF, name="w_sb")
            for m in (4, 5, 0, 1, 2, 3):  # match proj consumption order
                for hf in range(2):
                    nc.gpsimd.dma_start(
                        w_sb[:, m, ts(hf, 8)],
                        wqkv_d[m, :, ts(hf, 1024)].rearrange(
                            "p (o c) -> p o c", c=128))
            cos_sb = consts.tile([128, S], BF, name="cos_sb")
            nc.gpsimd.dma_start(cos_sb, cos_d)
            sin_sb = consts.tile([128, S], BF, name="sin_sb")
            nc.gpsimd.dma_start(sin_sb, sin_d)
            trim_sb = consts.tile([128, 128], BF, name="trim_sb")
            nc.gpsimd.dma_start(trim_sb, trim_d)
            onesc_sb = consts.tile([128, 128], BF, name="onesc_sb")
            nc.gpsimd.dma_start(onesc_sb, onesc_d)

            # PE warmup: keep the tensor engine busy from engine-start (the
            # memset source has no DMA dependency) so the HAM clock-gate
            # opens (1.2->2.4GHz) before real matmuls arrive, and stays open
            # across the initial xt/w DMA wait.
            warm_src = consts.tile([128, 128], BF, name="warm_src")
            nc.vector.memset(warm_src, 0.0)
            warm_ps = psB.tile([128, 128], F32, tag="psB", name="warm")
            for _ in range(60):
                nc.tensor.matmul(warm_ps, lhsT=warm_src, rhs=warm_src,
                                 start=True, stop=True)

            qt_sb = consts.tile([128, NH, S], BF, name="qt_sb")   # Q^T, rope'd
            kt_sb = consts.tile([128, S], BF, name="kt_sb")       # K^T, rope'd
            v_sb = consts.tile([128, ND, HD], BF, name="v_sb")    # V [tok, hd] blocks

            ag_in = [[dram.tile([256, 512], BF, name=f"agin{i}_{p}")
                      for p in range(2)] for i in range(NT)]
            ag_out = [[dram.tile([1024, 512], BF, name=f"agout{i}_{p}")
                       for p in range(2)] for i in range(NT)]

            def proj_chunk(tc_i):
                xt_t = io.tile([128, ND, 512], BF, tag="io512", name="xt_t")
                if tc_i == 0:
                    # fine-grained d-pair DMAs alternating sync/scalar so the
                    # d-ordered proj consumption is never starved at startup
                    for q in range(8):
                        eng = nc.sync if q % 2 == 0 else nc.scalar
                        eng.dma_start(xt_t[:, 2 * q:2 * (q + 1), :],
                                      xt_r[:, 2 * q:2 * (q + 1), ts(tc_i, 512)])
                else:
                    for q in range(4):
                        nc.sync.dma_start(
                            xt_t[:, 4 * q:4 * (q + 1), :],
                            xt_r[:, 4 * q:4 * (q + 1), ts(tc_i, 512)])
                for m in (4, 5, 0, 1, 2, 3):  # K, V first: their RoPE/transpose
                    # chains overlap the Q projections, so attention never
                    # waits on kt/v_sb
                    ps = psA.tile([128, 512], F32, tag="psA", name="ps_proj")
                    for d in range(ND):
                        nc.tensor.matmul(
                            ps, lhsT=w_sb[:, m, d, :], rhs=xt_t[:, d, :],
                            start=(d == 0), stop=(d == ND - 1),
                        )
                    if m < 5:
                        # RoPE in the de-interleaved basis:
                        #   out = ps*cos + crossswap(ps)*sin_signed
                        # crossswap reads ps at partition offset +-64.
                        t1 = work.tile([128, 512], F32, tag="rope_t1", name="t1")
                        nc.vector.tensor_tensor(
                            t1, ps, cos_sb[:, ts(tc_i, 512)], mybir.AluOpType.mult)
                        t2 = work.tile([128, 512], F32, tag="rope_t2", name="t2")
                        nc.vector.tensor_tensor(
                            t2[:64], ps[64:], sin_sb[:64, ts(tc_i, 512)],
                            mybir.AluOpType.mult)
                        nc.vector.tensor_tensor(
                            t2[64:], ps[:64], sin_sb[64:, ts(tc_i, 512)],
                            mybir.AluOpType.mult)
                        dst = (qt_sb[:, m, ts(tc_i, 512)] if m < 4
                               else kt_sb[:, ts(tc_i, 512)])
                        nc.vector.tensor_tensor(dst, t1, t2, mybir.AluOpType.add)
                    else:
                        # V^T chunk -> bf16 -> transpose to [tok, hd] blocks
                        vraw = work.tile([128, 512], BF, tag="rope_raw", name="vraw")
                        nc.scalar.copy(vraw, ps)
                        for j in range(4):
                            pst = psB.tile([128, 128], BF, tag="psB", name="ps_vT")
                            nc.tensor.transpose(pst, vraw[:, ts(j, 128)], ident_sb)
                            nc.vector.tensor_copy(v_sb[:, 4 * tc_i + j, :], pst)

            def attn_chunk(qc):
                for h in range(NH):
                    ps_att = psB.tile([128, 512], F32, tag="psB", name="ps_att")
                    ps_den = psA.tile([128, 512], F32, tag="psA", name="ps_den")
                    nkb = 4 * qc + 4
                    for kb in range(nkb):
                        r = kb - 4 * qc
                        o = max(r, 0) * 128   # first q column this kb can see
                        ps_s = psS.tile([128, 512], F32, tag="psS", name="ps_s")
                        nc.tensor.matmul(
                            ps_s[:, o:], lhsT=kt_sb[:, ts(kb, 128)],
                            rhs=qt_sb[:, h, 512 * qc + o:512 * (qc + 1)],
                            start=True, stop=(r < 0))
                        if r >= 0:
                            # causal mask folded into the scores: -30/SCALE
                            # bias on the strictly-masked diagonal square;
                            # exp(s-30) ~ 0. Keeps DVE off the exp->pV path.
                            nc.tensor.matmul(
                                ps_s[:, o:o + 128], lhsT=trim_sb,
                                rhs=ident_sb, start=False, stop=True)
                        pt = work.tile([128, 512], BF, tag="pt", name="pt",
                                       bufs=4)
                        nc.scalar.activation(
                            pt[:, o:], ps_s[:, o:],
                            mybir.ActivationFunctionType.Exp, scale=SCALE)
                        nc.tensor.matmul(
                            ps_att[:, o:], lhsT=v_sb[:, kb, :], rhs=pt[:, o:],
                            start=(kb == 0), stop=(kb == nkb - 1))
                        nc.tensor.matmul(
                            ps_den[:, o:], lhsT=onesc_sb, rhs=pt[:, o:],
                            start=(kb == 0), stop=(kb == nkb - 1))
                    # ones[128,128] lhsT made ps_den the partition-broadcast den
                    bden = work.tile([128, 512], F32, tag="bden", name="bden")
                    nc.vector.reciprocal_approx_fast(bden, ps_den)
                    att = work.tile([128, 512], BF, tag="att", name="att")
                    nc.vector.tensor_tensor(att, ps_att, bden,
                                            mybir.AluOpType.mult)
                    nc.scalar.dma_start(ag_in[qc][h // 2][ts(h % 2, 128), :], att)
                    if h % 2 == 1:
                        nc.gpsimd.collective_compute(
                            "AllGather", mybir.AluOpType.bypass,
                            replica_groups=RG,
                            ins=[ag_in[qc][h // 2][:].opt()],
                            outs=[ag_out[qc][h // 2][:].opt()])

            def oproj_chunk(tc_i):
                rhs = io.tile([128, ND, 512], BF, tag="io512", name="oproj_rhs")
                nc.sync.dma_start(
                    rhs[:, :8, :],
                    ag_out[tc_i][0].rearrange("(o p) t -> p o t", p=128))
                nc.sync.dma_start(
                    rhs[:, 8:, :],
                    ag_out[tc_i][1].rearrange("(o p) t -> p o t", p=128))
                for j in range(4):
                    if tc_i == NT - 1 and j == 3:
                        # split the very last output tile in two so the copy
                        # and store of the first half overlap the second
                        # half's matmuls (shorter kernel tail)
                        for hf in range(2):
                            ps_o = psA.tile([128, 256], F32, tag="psA",
                                            name="ps_oh")
                            for c in range(ND):
                                nc.tensor.matmul(
                                    ps_o, lhsT=woT_sb[:, c, ts(j, 128)],
                                    rhs=rhs[:, c, ts(hf, 256)],
                                    start=(c == 0), stop=(c == ND - 1))
                            o32h = work.tile([128, 256], F32, tag="o32",
                                             name="o32h")
                            nc.vector.tensor_copy(o32h, ps_o)
                            nc.sync.dma_start(
                                out_d[ts(j, 128), 512 * tc_i + 256 * hf:
                                      512 * tc_i + 256 * (hf + 1)], o32h)
                        continue
                    ps_o = psA.tile([128, 512], F32, tag="psA", name="ps_o")
                    for c in range(ND):
                        nc.tensor.matmul(
                            ps_o, lhsT=woT_sb[:, c, ts(j, 128)], rhs=rhs[:, c, :],
                            start=(c == 0), stop=(c == ND - 1))
                    o32 = work.tile([128, 512], F32, tag="o32", name="o32")
                    nc.vector.tensor_copy(o32, ps_o)
                    nc.sync.dma_start(out_d[ts(j, 128), ts(tc_i, 512)], o32)

            for i in range(NT):
                proj_chunk(i)
                attn_chunk(i)
            woT_sb = consts.tile([128, ND, 512], BF, name="woT_sb")
            nc.gpsimd.dma_start(woT_sb, woT_r)
            for i in range(NT):
                oproj_chunk(i)

    nc.compile()
    return nc


def make_in_maps(x, freqs_cos, freqs_sin, wq, wk, wv, wo):
    fc = np.asarray(freqs_cos, np.float32)
    fs = np.asarray(freqs_sin, np.float32)
    # De-interleaved RoPE basis: rows 0-63 real lanes, 64-127 imag lanes.
    cos_exp = np.concatenate([fc.T, fc.T], axis=0).astype(bf16)       # [128, S]
    sin_sgn = np.concatenate([-fs.T, fs.T], axis=0).astype(bf16)      # [128, S]
    trim = (np.triu(np.ones((128, 128), np.float32), 1) * (-30.0 / SCALE)).astype(bf16)
    ident = np.eye(128, dtype=np.float32).astype(bf16)
    onesc = np.ones((128, 128), np.float32).astype(bf16)

    # per-head row permutation: [r0,i0,r1,i1,...] -> [r0..r63, i0..i63]
    deint = np.concatenate([np.arange(0, 128, 2), np.arange(1, 128, 2)])

    xt = [np.ascontiguousarray(np.asarray(x[b], np.float32).T).astype(bf16)
          for b in range(B)]
    wq_f = np.asarray(wq, np.float32)
    wk_f = np.asarray(wk, np.float32)
    wv_f = np.asarray(wv, np.float32)
    in_maps = []
    for core in range(8):
        b, g = divmod(core, 4)
        wq_g = wq_f[512 * g:512 * (g + 1)].reshape(4, 128, D)[:, deint, :]
        wq_g = wq_g.reshape(512, D)
        wk_g = wk_f[128 * g:128 * (g + 1)][deint, :]
        wqkvT = np.concatenate(
            [wq_g.T, wk_g.T, wv_f[128 * g:128 * (g + 1)].T], axis=1)
        # m-major SBUF-order blocks: [6][p 128][o*128+c 2048]
        wqkvT = np.ascontiguousarray(
            wqkvT.reshape(16, 128, 768).transpose(2, 1, 0)   # [768 m, 128 p, 16 o]
        )  # temp
        wqkvT = np.ascontiguousarray(np.stack(
            [wqkvT[128 * m:128 * (m + 1)].transpose(1, 2, 0).reshape(128, 2048)
             for m in range(6)]))
        order = [0, 1, 4, 5, 8, 9, 12, 13, 2, 3, 6, 7, 10, 11, 14, 15]
        woT = np.asarray(wo, np.float32)[512 * g:512 * (g + 1), :].T
        woT = woT.reshape(16, 128, 512)[order].reshape(2048, 512)
        in_maps.append({
            "xt": xt[b],
            "wqkvT": np.ascontiguousarray(wqkvT).astype(bf16),
            "woT": np.ascontiguousarray(woT).astype(bf16),
            "cose": cos_exp,
            "sins": sin_sgn,
            "trim": trim,
            "ident": ident,
            "onesc": onesc,
        })
    return in_maps


_NC = None


def get_nc():
    global _NC
    if _NC is None:
        _NC = build_nc()
    return _NC


def assemble_out(results):
    out = np.zeros((B, S, D), np.float32)
    for core in range(8):
        b, g = divmod(core, 4)
        out[b, :, 512 * g:512 * (g + 1)] = results[core]["out"].T
    return out


def kernel(x, freqs_cos, freqs_sin, wq, wk, wv, wo):
    import os
    os.environ.setdefault("BASS_NEVER_TRACE", "1")  # NTFF hook absent headless
    nc = get_nc()
    in_maps = make_in_maps(x, freqs_cos, freqs_sin, wq, wk, wv, wo)
    res = run_bass_kernel_spmd(nc, in_maps, core_ids=list(range(8)))
    return assemble_out(res.results)
